# revision 13
# baseline (speedup 1.0000x reference)
"""Trainium2 Bass kernel for nn_MultiHeadAttentionBlock_49967649521921.

Reference computation (per batch b, x viewed as [C=512, N=1024]):
    q = Wq @ x ; k = Wk @ x ; v = Wv @ x          (1x1 convs, biases are zeros)
    per head h (8 heads, hd=64):
      scores[d,e] = sum_n q_h[d,n] k_h[e,n] / 8
      attn = softmax(scores, axis=e)
      out_h[d,n]  = sum_e attn[d,e] v_h[e,n]
    y[c',s'] = out[h, d, n] with c' = h*64 + n//16, s' = (n%16)*64 + d
    final = Wo @ y    -> reshape [512, 32, 32]

Sharding: data-parallel over batch. 16 batches / 8 cores = 2 per core.
No collectives; host scatters inputs and gathers outputs.

v2 design (all matmul operands bf16; PSUM accumulates f32):
  - Host permutes the spatial axis n = 16a + r -> m = 64r + a. Everything
    up to attn@v is order-agnostic in n (projections broadcast over n;
    scores contract over n), and in the m-order the reference's
    transpose(2,3).reshape scramble becomes plain strided SBUF copies:
      po[m=(rr,a), dd=(hh,d)] slices map directly onto
      y[c'=(hh,a), s=(r,d)] slices with r = 2*ncn + rr.
  - scores computed per head-PAIR: [128 (2h d), 128 (2h e)] tiles (the
    off-diagonal cross-head quadrants are computed and discarded; this
    keeps the moving dim at 128 so LDWEIGHTS pipelines fully).
  - softmax entirely off the tensor engine: exp on the Act engine with
    accum_out giving the row sums for free, DVE reciprocal, and a
    Copy-activation with a per-partition scale AP for the normalize.
  - attn@v uses a block-diagonal [128,128] attn tile per pair (built by
    8 small PE transposes) so both heads' outputs come from one matmul
    with contraction 128.
  - phase order across the two batches hides every DMA behind matmuls:
      qk0 s10 v0 T0 av0 | qk1 s11 f0 v1 T1 av1 f1
"""

import os
import sys

import numpy as np

for _p in ("/opt/trn_rl_repo",):
    if _p not in sys.path and os.path.isdir(_p):
        sys.path.insert(0, _p)

from contextlib import ExitStack

import concourse.bass as bass
import concourse.tile as tile
from concourse import bacc
from concourse import mybir
from concourse.bass_utils import run_bass_kernel_spmd

F32 = mybir.dt.float32
BF16 = mybir.dt.bfloat16
AF = mybir.ActivationFunctionType

N_CORES = 8
B_PER_CORE = 2
C = 512
N = 1024
NH = 8
HD = 64


def _split_excess_dma_waits(nc):
    """walrus' static-DMA (PSEUDO_DMA_DIRECT2D) encoding accepts a single
    sync-wait; Bacc's generate_event_semaphores only splits waits on compute
    instructions. Move excess DMA waits onto preceding EventSemaphore
    carriers (2 waits each) on the same engine queue."""
    for f in nc.m.functions:
        for blk in f.blocks:
            changed = False
            new_insts = []
            for inst in blk.instructions:
                si = inst.sync_info
                waits = list(si.on_wait) if si is not None and si.on_wait else []
                if inst.opcode == "DMACopy" and len(waits) > 1:
                    keep, excess = waits[:1], waits[1:]
                    k = 0
                    while excess:
                        chunk, excess = excess[:2], excess[2:]
                        ev = mybir.InstEventSemaphore(
                            name=f"{inst.name}-evw{k}",
                            opcode="EventSemaphore",
                            engine=inst.engine,
                            sync_info=mybir.SyncInfo(on_wait=chunk, on_update=[]),
                        )
                        new_insts.append(ev)
                        k += 1
                    inst.sync_info = mybir.SyncInfo(
                        on_wait=keep, on_update=list(si.on_update or [])
                    )
                    changed = True
                new_insts.append(inst)
            if changed:
                blk.instructions = new_insts


def build_program():
    nc = bacc.Bacc("TRN2", target_bir_lowering=False, debug=False)

    # all inputs pre-laid-out on the host so each DMA is a straight
    # [128, big-contiguous] copy (128 descriptors, cheap trigger)
    x_d = nc.dram_tensor("x", [B_PER_CORE, 128, 4, N], BF16, kind="ExternalInput").ap()
    wq_d = nc.dram_tensor("wqt", [128, 4, C], BF16, kind="ExternalInput").ap()
    wk_d = nc.dram_tensor("wkt", [128, 4, C], BF16, kind="ExternalInput").ap()
    wv_d = nc.dram_tensor("wvt", [128, 4, C], BF16, kind="ExternalInput").ap()
    wo_d = nc.dram_tensor("wot", [128, 4, C], BF16, kind="ExternalInput").ap()
    out_d = nc.dram_tensor(
        "out", [B_PER_CORE, 128, 2, 4, 512], BF16, kind="ExternalOutput"
    ).ap()

    with tile.TileContext(nc) as tc, ExitStack() as ctx:
        wp = ctx.enter_context(tc.tile_pool(name="w", bufs=1))
        xp = ctx.enter_context(tc.tile_pool(name="x", bufs=2))
        qkp = ctx.enter_context(tc.tile_pool(name="qk", bufs=2))
        vp = ctx.enter_context(tc.tile_pool(name="v", bufs=2))
        smp = ctx.enter_context(tc.tile_pool(name="sm", bufs=2))
        yp = ctx.enter_context(tc.tile_pool(name="y", bufs=2))
        ogp = ctx.enter_context(tc.tile_pool(name="og", bufs=3))

        ps_big = ctx.enter_context(tc.tile_pool(name="psb", bufs=4, space="PSUM"))
        ps_s1 = ctx.enter_context(tc.tile_pool(name="pss", bufs=1, space="PSUM"))
        ps_av = ctx.enter_context(tc.tile_pool(name="psa", bufs=3, space="PSUM"))

        w_sb = {}

        def _load_w(name, d, eng):
            t = wp.tile([128, 4, C], BF16, tag=name, name=f"w_{name}")
            eng.dma_start(t[:, :, :], d[:, :, :])
            w_sb[name] = t

        st = [{} for _ in range(B_PER_CORE)]

        def s_load(b, e0, e1):
            x_sb = xp.tile([128, 4, N], BF16, tag="xsb", name=f"x_sb{b}")
            e0.dma_start(x_sb[:, 0:2, :], x_d[b, :, 0:2, :])
            e1.dma_start(x_sb[:, 2:4, :], x_d[b, :, 2:4, :])
            st[b]["x"] = x_sb

        def s_proj_qk(b):
            x_sb = st[b]["x"]
            qt_sb = qkp.tile([128, 8, C], BF16, tag="qt", name=f"qt{b}")
            kt_sb = qkp.tile([128, 8, C], BF16, tag="kt", name=f"kt{b}")
            st[b]["qt"], st[b]["kt"] = qt_sb, kt_sb
            hook = st[b].pop("qk_hook", None)
            for wname, t_sb, ceng in (("wq", qt_sb, "v"), ("wk", kt_sb, "s")):
                for ncn in range(8):
                    msl = slice(ncn * 128, (ncn + 1) * 128)
                    pq = ps_big.tile([128, C], F32, tag="big", name=f"p{wname}{b}_{ncn}")
                    for cc in range(4):
                        nc.tensor.matmul(
                            pq[:, :], x_sb[:, cc, msl], w_sb[wname][:, cc, :],
                            start=(cc == 0), stop=(cc == 3),
                        )
                    if ceng == "v":
                        nc.vector.tensor_copy(t_sb[:, ncn, :], pq[:, :])
                    else:
                        nc.scalar.copy(t_sb[:, ncn, :], pq[:, :])
                    if hook is not None and wname == "wq":
                        hook(ncn, qt_sb)

        def s_scores(b):
            """per head-pair hp: s1[128 (2h d), 128 (2h e)] = qt^T kt; then
            softmax over e on scalar+vector; at_bd = block-diag attn^T."""
            qt_sb, kt_sb = st[b]["qt"], st[b]["kt"]
            ps1 = ps_s1.tile([128, 4, 128], F32, tag="s1", name=f"ps1_{b}")
            es = smp.tile([128, 4, HD], BF16, tag="es", name=f"es{b}")
            rs = smp.tile([128, 4, 1], F32, tag="rs", name=f"rs{b}")
            rcp = smp.tile([128, 4, 1], F32, tag="rcp", name=f"rcp{b}")
            at1 = smp.tile([128, 4, HD], BF16, tag="at1", name=f"at1_{b}")
            at_bd = smp.tile([128, 4, 128], BF16, tag="atbd", name=f"at_bd{b}")
            nc.vector.memset(at_bd[:, :, :], 0.0)
            for hp in range(4):
                csl = slice(hp * 128, (hp + 1) * 128)
                for ncn in range(8):
                    nc.tensor.matmul(
                        ps1[:, hp, :],
                        qt_sb[:, ncn, csl],
                        kt_sb[:, ncn, csl],
                        start=(ncn == 0), stop=(ncn == 7),
                    )
                for hh in range(2):
                    psl = slice(hh * 64, hh * 64 + 64)
                    nc.scalar.activation(
                        es[psl, hp, :], ps1[psl, hp, psl],
                        AF.Exp, scale=0.125,
                        accum_out=rs[psl, hp, :],
                    )
                nc.vector.reciprocal(rcp[:, hp, :], rs[:, hp, :])
                nc.scalar.activation(
                    at1[:, hp, :], es[:, hp, :], AF.Copy, scale=rcp[:, hp, :],
                )
            st[b]["at1"], st[b]["at_bd"] = at1, at_bd

        def s_at_transpose(b):
            at1, at_bd = st[b]["at1"], st[b]["at_bd"]
            for hp in range(4):
                for hh in range(2):
                    pb = hh * 64
                    for bi in range(2):
                        for bj in range(2):
                            nc.vector.transpose(
                                at_bd[
                                    pb + bi * 32 : pb + bi * 32 + 32,
                                    hp,
                                    pb + bj * 32 : pb + bj * 32 + 32,
                                ],
                                at1[
                                    pb + bj * 32 : pb + bj * 32 + 32,
                                    hp,
                                    bi * 32 : bi * 32 + 32,
                                ],
                            )

        def s_proj_v(b):
            x_sb = st[b]["x"]
            v_sb = vp.tile([128, 4, N], BF16, tag="vsb", name=f"v_sb{b}")
            for hp in range(4):
                if hp == 1:
                    s_at_transpose(b)
                for nh in range(2):
                    pv = ps_big.tile([128, C], F32, tag="big", name=f"pv{b}_{hp}_{nh}")
                    for cc in range(4):
                        nc.tensor.matmul(
                            pv[:, :],
                            w_sb["wv"][:, cc, hp * 128 : (hp + 1) * 128],
                            x_sb[:, cc, nh * 512 : (nh + 1) * 512],
                            start=(cc == 0), stop=(cc == 3),
                        )
                    if nh == 0:
                        nc.vector.tensor_copy(v_sb[:, hp, 0:512], pv[:, :])
                    else:
                        nc.scalar.copy(v_sb[:, hp, 512:1024], pv[:, :])
            st[b]["v"] = v_sb

        def s_attn_v_group(b, hp, sh):
            v_sb, at_bd = st[b]["v"], st[b]["at_bd"]
            y_sb = st[b]["y"]
            po = ps_av.tile([128, 4, 128], F32, tag="po", name=f"po{b}_{hp}_{sh}")
            for j in range(4):
                ncn = 4 * sh + j
                nc.tensor.matmul(
                    po[:, j, :],
                    v_sb[:, hp, ncn * 128 : (ncn + 1) * 128],
                    at_bd[:, hp, :],
                    start=True, stop=True,
                )
            k = 0
            for rr in range(2):
                for hh in range(2):
                    dst = y_sb[hh * 64 : hh * 64 + 64, hp, sh, :, rr, :]
                    srcp = po[rr * 64 : rr * 64 + 64, :, hh * 64 : hh * 64 + 64]
                    if k % 2 == 0:
                        nc.vector.tensor_copy(dst, srcp)
                    else:
                        nc.scalar.copy(dst, srcp)
                    k += 1

        def s_attn_v(b, skip_last=False):
            """po[m, dd] per (hp, ncn); y copies realize the reshape."""
            # y layout [128 (hh a), cc=hp, sh, j, rr, d]: s = r*64+d with
            # r = 8*sh + 2*j + rr
            y_sb = yp.tile([128, 4, 2, 4, 2, HD], BF16, tag="ysb", name=f"y_sb{b}")
            st[b]["y"] = y_sb
            for hp in range(4):
                for sh in range(2):
                    if skip_last and hp == 3 and sh == 1:
                        continue
                    s_attn_v_group(b, hp, sh)

        def s_final(b, shs=(0, 1)):
            y_sb = st[b]["y"]
            for sh in shs:
                og = ogp.tile([128, 4, 512], BF16, tag="og", name=f"og{b}_{sh}")
                for oc in range(4):
                    pf = ps_big.tile([128, C], F32, tag="big", name=f"pf{b}_{oc}_{sh}")
                    for cp in range(4):
                        nc.tensor.matmul(
                            pf[:, :],
                            w_sb["wo"][:, cp, oc * 128 : (oc + 1) * 128],
                            y_sb[:, cp, sh, :, :, :],
                            start=(cp == 0), stop=(cp == 3),
                        )
                    nc.vector.tensor_copy(og[:, oc, 0:256], pf[:, 0:256])
                    nc.scalar.copy(og[:, oc, 256:512], pf[:, 256:512])
                nc.sync.dma_start(out_d[b, :, sh, :, :], og[:, :, :])

        # ---- schedule ----
        # descriptors from all engines share the 16 DMA queues in trigger
        # order, so the early window must carry ONLY wq/wk/ident/x0; the
        # wv/wo/x1 loads are deferred by 1-element gpsimd token copies that
        # depend on qt chunks (the DMA dst overlaps the token write, so the
        # trigger inherits the dependency).
        s_load(0, nc.sync, nc.scalar)
        _load_w("wq", wq_d, nc.sync)
        _load_w("wk", wk_d, nc.sync)

        wv_t = wp.tile([128, 4, C], BF16, tag="wv", name="w_wv")
        wo_t = wp.tile([128, 4, C], BF16, tag="wo", name="w_wo")
        w_sb["wv"], w_sb["wo"] = wv_t, wo_t
        x1_sb = xp.tile([128, 4, N], BF16, tag="xsb", name="x_sb1")
        st[1]["x"] = x1_sb

        def qk0_hook(ncn, qt_sb):
            if ncn == 0:
                nc.gpsimd.tensor_copy(wv_t[0:1, 0, 0:1], qt_sb[0:1, 0, 0:1])
                nc.gpsimd.dma_start(wv_t[:, :, :], wv_d[:, :, :])
            elif ncn == 2:
                nc.gpsimd.tensor_copy(wo_t[0:1, 0, 0:1], qt_sb[0:1, 2, 0:1])
                nc.gpsimd.dma_start(wo_t[:, :, :], wo_d[:, :, :])
            elif ncn == 4:
                nc.gpsimd.tensor_copy(x1_sb[0:1, 0, 0:1], qt_sb[0:1, 4, 0:1])
                nc.gpsimd.dma_start(x1_sb[:, :, :], x_d[1, :, :, :])

        st[0]["qk_hook"] = qk0_hook
        s_proj_qk(0)
        s_scores(0)
        s_proj_v(0)          # T(0) interleaved at hp==1
        s_attn_v(0)
        s_proj_qk(1)
        s_scores(1)
        s_final(0)
        s_proj_v(1)          # T(1) interleaved at hp==1
        s_attn_v(1, skip_last=True)
        s_final(1, shs=(0,))
        s_attn_v_group(1, 3, 1)
        s_final(1, shs=(1,))

    nc.compile()
    _split_excess_dma_waits(nc)
    return nc


_PROGRAM = None


def _get_program():
    global _PROGRAM
    if _PROGRAM is None:
        _PROGRAM = build_program()
    return _PROGRAM


def make_in_maps(x, Wq, Wk, Wv, Wo):
    import ml_dtypes

    bf = ml_dtypes.bfloat16
    # permute spatial axis n = 16a + r -> m = 64r + a, then lay out as
    # [128 partition, 4 cc, N] so the device DMA is a straight copy
    xm = (
        x.reshape(16, C, 64, 16)
        .transpose(0, 1, 3, 2)
        .reshape(16, 4, 128, N)
        .transpose(0, 2, 1, 3)
        .astype(bf)
    )

    def _w(W):
        return np.ascontiguousarray(
            W.T.reshape(4, 128, C).transpose(1, 0, 2).astype(bf)
        )

    wqt, wkt, wvt, wot = _w(Wq), _w(Wk), _w(Wv), _w(Wo)
    in_maps = []
    for c in range(N_CORES):
        in_maps.append(
            {
                "x": np.ascontiguousarray(xm[c * B_PER_CORE : (c + 1) * B_PER_CORE]),
                "wqt": wqt,
                "wkt": wkt,
                "wvt": wvt,
                "wot": wot,
            }
        )
    return in_maps


def kernel(x, Wq, bq, Wk, bk, Wv, bv, Wo, bo, _trace=False):
    # biases are zeros by construction in this problem (spec fill="zeros");
    # they are not applied on-device.
    nc = _get_program()
    in_maps = make_in_maps(x, Wq, Wk, Wv, Wo)
    res = run_bass_kernel_spmd(nc, in_maps, list(range(N_CORES)), trace=_trace)
    outs = [
        np.asarray(res.results[c]["out"]).astype(np.float32) for c in range(N_CORES)
    ]
    # out_r [b, p, sh, oc, s'] -> F[b, oc*128+p, sh*512+s']
    full = (
        np.concatenate(outs, axis=0)
        .transpose(0, 3, 1, 2, 4)
        .reshape(16, C, N)
        .reshape(16, C, 32, 32)
    )
    if _trace:
        return full, res
    return full


# revision 14
# speedup vs baseline: 1.0373x; 1.0373x over previous
"""Trainium2 Bass kernel for nn_MultiHeadAttentionBlock_49967649521921.

Reference computation (per batch b, x viewed as [C=512, N=1024]):
    q = Wq @ x ; k = Wk @ x ; v = Wv @ x          (1x1 convs, biases are zeros)
    per head h (8 heads, hd=64):
      scores[d,e] = sum_n q_h[d,n] k_h[e,n] / 8
      attn = softmax(scores, axis=e)
      out_h[d,n]  = sum_e attn[d,e] v_h[e,n]
    y[c',s'] = out[h, d, n] with c' = h*64 + n//16, s' = (n%16)*64 + d
    final = Wo @ y    -> reshape [512, 32, 32]

Sharding: data-parallel over batch. 16 batches / 8 cores = 2 per core.
No collectives; host scatters inputs and gathers outputs.

v2 design (all matmul operands bf16; PSUM accumulates f32):
  - Host permutes the spatial axis n = 16a + r -> m = 64r + a. Everything
    up to attn@v is order-agnostic in n (projections broadcast over n;
    scores contract over n), and in the m-order the reference's
    transpose(2,3).reshape scramble becomes plain strided SBUF copies:
      po[m=(rr,a), dd=(hh,d)] slices map directly onto
      y[c'=(hh,a), s=(r,d)] slices with r = 2*ncn + rr.
  - scores computed per head-PAIR: [128 (2h d), 128 (2h e)] tiles (the
    off-diagonal cross-head quadrants are computed and discarded; this
    keeps the moving dim at 128 so LDWEIGHTS pipelines fully).
  - softmax entirely off the tensor engine: exp on the Act engine with
    accum_out giving the row sums for free, DVE reciprocal, and a
    Copy-activation with a per-partition scale AP for the normalize.
  - attn@v uses a block-diagonal [128,128] attn tile per pair (built by
    8 small PE transposes) so both heads' outputs come from one matmul
    with contraction 128.
  - phase order across the two batches hides every DMA behind matmuls:
      qk0 s10 v0 T0 av0 | qk1 s11 f0 v1 T1 av1 f1
"""

import os
import sys

import numpy as np

for _p in ("/opt/trn_rl_repo",):
    if _p not in sys.path and os.path.isdir(_p):
        sys.path.insert(0, _p)

from contextlib import ExitStack

import concourse.bass as bass
import concourse.tile as tile
from concourse import bacc
from concourse import mybir
from concourse.bass_utils import run_bass_kernel_spmd

F32 = mybir.dt.float32
BF16 = mybir.dt.bfloat16
AF = mybir.ActivationFunctionType

N_CORES = 8
B_PER_CORE = 2
C = 512
N = 1024
NH = 8
HD = 64


def _split_excess_dma_waits(nc):
    """walrus' static-DMA (PSEUDO_DMA_DIRECT2D) encoding accepts a single
    sync-wait; Bacc's generate_event_semaphores only splits waits on compute
    instructions. Move excess DMA waits onto preceding EventSemaphore
    carriers (2 waits each) on the same engine queue."""
    for f in nc.m.functions:
        for blk in f.blocks:
            changed = False
            new_insts = []
            for inst in blk.instructions:
                si = inst.sync_info
                waits = list(si.on_wait) if si is not None and si.on_wait else []
                if inst.opcode == "DMACopy" and len(waits) > 1:
                    keep, excess = waits[:1], waits[1:]
                    k = 0
                    while excess:
                        chunk, excess = excess[:2], excess[2:]
                        ev = mybir.InstEventSemaphore(
                            name=f"{inst.name}-evw{k}",
                            opcode="EventSemaphore",
                            engine=inst.engine,
                            sync_info=mybir.SyncInfo(on_wait=chunk, on_update=[]),
                        )
                        new_insts.append(ev)
                        k += 1
                    inst.sync_info = mybir.SyncInfo(
                        on_wait=keep, on_update=list(si.on_update or [])
                    )
                    changed = True
                new_insts.append(inst)
            if changed:
                blk.instructions = new_insts


def build_program():
    nc = bacc.Bacc("TRN2", target_bir_lowering=False, debug=False)

    # all inputs pre-laid-out on the host so each DMA is a straight
    # [128, big-contiguous] copy (128 descriptors, cheap trigger)
    x_d = nc.dram_tensor("x", [B_PER_CORE, 128, 4, N], BF16, kind="ExternalInput").ap()
    wq_d = nc.dram_tensor("wqt", [128, 4, C], BF16, kind="ExternalInput").ap()
    wk_d = nc.dram_tensor("wkt", [128, 4, C], BF16, kind="ExternalInput").ap()
    wv_d = nc.dram_tensor("wvt", [128, 4, C], BF16, kind="ExternalInput").ap()
    wo_d = nc.dram_tensor("wot", [128, 4, C], BF16, kind="ExternalInput").ap()
    id_d = nc.dram_tensor("ident", [128, HD], BF16, kind="ExternalInput").ap()
    out_d = nc.dram_tensor(
        "out", [B_PER_CORE, 128, 2, 4, 512], BF16, kind="ExternalOutput"
    ).ap()

    with tile.TileContext(nc) as tc, ExitStack() as ctx:
        wp = ctx.enter_context(tc.tile_pool(name="w", bufs=1))
        xp = ctx.enter_context(tc.tile_pool(name="x", bufs=2))
        qkp = ctx.enter_context(tc.tile_pool(name="qk", bufs=2))
        vp = ctx.enter_context(tc.tile_pool(name="v", bufs=2))
        smp = ctx.enter_context(tc.tile_pool(name="sm", bufs=2))
        yp = ctx.enter_context(tc.tile_pool(name="y", bufs=2))
        ogp = ctx.enter_context(tc.tile_pool(name="og", bufs=3))

        ps_big = ctx.enter_context(tc.tile_pool(name="psb", bufs=3, space="PSUM"))
        ps_s1 = ctx.enter_context(tc.tile_pool(name="pss", bufs=1, space="PSUM"))
        ps_tr = ctx.enter_context(tc.tile_pool(name="pst", bufs=1, space="PSUM"))
        ps_av = ctx.enter_context(tc.tile_pool(name="psa", bufs=3, space="PSUM"))

        w_sb = {}

        def _load_w(name, d, eng):
            t = wp.tile([128, 4, C], BF16, tag=name, name=f"w_{name}")
            eng.dma_start(t[:, :, :], d[:, :, :])
            w_sb[name] = t

        st = [{} for _ in range(B_PER_CORE)]

        def s_load(b, e0, e1):
            x_sb = xp.tile([128, 4, N], BF16, tag="xsb", name=f"x_sb{b}")
            e0.dma_start(x_sb[:, 0:2, :], x_d[b, :, 0:2, :])
            e1.dma_start(x_sb[:, 2:4, :], x_d[b, :, 2:4, :])
            st[b]["x"] = x_sb

        def s_proj_qk(b):
            x_sb = st[b]["x"]
            qt_sb = qkp.tile([128, 8, C], BF16, tag="qt", name=f"qt{b}")
            kt_sb = qkp.tile([128, 8, C], BF16, tag="kt", name=f"kt{b}")
            st[b]["qt"], st[b]["kt"] = qt_sb, kt_sb
            hook = st[b].pop("qk_hook", None)
            for wname, t_sb, ceng in (("wq", qt_sb, "v"), ("wk", kt_sb, "s")):
                for ncn in range(8):
                    msl = slice(ncn * 128, (ncn + 1) * 128)
                    pq = ps_big.tile([128, C], F32, tag="big", name=f"p{wname}{b}_{ncn}")
                    for cc in range(4):
                        nc.tensor.matmul(
                            pq[:, :], x_sb[:, cc, msl], w_sb[wname][:, cc, :],
                            start=(cc == 0), stop=(cc == 3),
                        )
                    if ceng == "v":
                        nc.vector.tensor_copy(t_sb[:, ncn, :], pq[:, :])
                    else:
                        nc.scalar.copy(t_sb[:, ncn, :], pq[:, :])
                    if hook is not None and wname == "wq":
                        hook(ncn, qt_sb)

        def s_scores(b):
            """per head-pair hp: s1[128 (2h d), 128 (2h e)] = qt^T kt; then
            softmax over e on scalar+vector; at_bd = block-diag attn^T."""
            qt_sb, kt_sb = st[b]["qt"], st[b]["kt"]
            ps1 = ps_s1.tile([128, 4, 128], F32, tag="s1", name=f"ps1_{b}")
            es = smp.tile([128, 4, HD], BF16, tag="es", name=f"es{b}")
            rs = smp.tile([128, 4, 1], F32, tag="rs", name=f"rs{b}")
            rcp = smp.tile([128, 4, 1], F32, tag="rcp", name=f"rcp{b}")
            at1 = smp.tile([128, 4, HD], BF16, tag="at1", name=f"at1_{b}")
            at_bd = smp.tile([128, 4, 128], BF16, tag="atbd", name=f"at_bd{b}")
            nc.vector.memset(at_bd[:, :, :], 0.0)
            for hp in range(4):
                csl = slice(hp * 128, (hp + 1) * 128)
                for ncn in range(8):
                    nc.tensor.matmul(
                        ps1[:, hp, :],
                        qt_sb[:, ncn, csl],
                        kt_sb[:, ncn, csl],
                        start=(ncn == 0), stop=(ncn == 7),
                    )
                for hh in range(2):
                    psl = slice(hh * 64, hh * 64 + 64)
                    nc.scalar.activation(
                        es[psl, hp, :], ps1[psl, hp, psl],
                        AF.Exp, scale=0.125,
                        accum_out=rs[psl, hp, :],
                    )
                nc.vector.reciprocal(rcp[:, hp, :], rs[:, hp, :])
                nc.scalar.activation(
                    at1[:, hp, :], es[:, hp, :], AF.Copy, scale=rcp[:, hp, :],
                )
            st[b]["at1"], st[b]["at_bd"] = at1, at_bd

        def s_at_transpose(b):
            at1, at_bd = st[b]["at1"], st[b]["at_bd"]
            pst = ps_tr.tile([64, 8, HD], BF16, tag="tr", name=f"pst{b}")
            for hp in range(4):
                for hh in range(2):
                    h = 2 * hp + hh
                    psl = slice(hh * 64, hh * 64 + 64)
                    nc.tensor.transpose(
                        pst[:, h, :], at1[psl, hp, :], ident[psl, :]
                    )
                    if hh == 0:
                        nc.vector.tensor_copy(at_bd[psl, hp, psl], pst[:, h, :])
                    else:
                        nc.scalar.copy(at_bd[psl, hp, psl], pst[:, h, :])

        def s_proj_v(b):
            x_sb = st[b]["x"]
            v_sb = vp.tile([128, 4, N], BF16, tag="vsb", name=f"v_sb{b}")
            for hp in range(4):
                if hp == 1:
                    s_at_transpose(b)
                for nh in range(2):
                    pv = ps_big.tile([128, C], F32, tag="big", name=f"pv{b}_{hp}_{nh}")
                    for cc in range(4):
                        nc.tensor.matmul(
                            pv[:, :],
                            w_sb["wv"][:, cc, hp * 128 : (hp + 1) * 128],
                            x_sb[:, cc, nh * 512 : (nh + 1) * 512],
                            start=(cc == 0), stop=(cc == 3),
                        )
                    if nh == 0:
                        nc.vector.tensor_copy(v_sb[:, hp, 0:512], pv[:, :])
                    else:
                        nc.scalar.copy(v_sb[:, hp, 512:1024], pv[:, :])
            st[b]["v"] = v_sb

        def s_attn_v_group(b, hp, sh):
            v_sb, at_bd = st[b]["v"], st[b]["at_bd"]
            y_sb = st[b]["y"]
            po = ps_av.tile([128, 4, 128], F32, tag="po", name=f"po{b}_{hp}_{sh}")
            for j in range(4):
                ncn = 4 * sh + j
                nc.tensor.matmul(
                    po[:, j, :],
                    v_sb[:, hp, ncn * 128 : (ncn + 1) * 128],
                    at_bd[:, hp, :],
                    start=True, stop=True,
                )
            k = 0
            for rr in range(2):
                for hh in range(2):
                    dst = y_sb[hh * 64 : hh * 64 + 64, hp, sh, :, rr, :]
                    srcp = po[rr * 64 : rr * 64 + 64, :, hh * 64 : hh * 64 + 64]
                    if k % 2 == 0:
                        nc.vector.tensor_copy(dst, srcp)
                    else:
                        nc.scalar.copy(dst, srcp)
                    k += 1

        def s_attn_v(b, skip_last=False):
            """po[m, dd] per (hp, ncn); y copies realize the reshape."""
            # y layout [128 (hh a), cc=hp, sh, j, rr, d]: s = r*64+d with
            # r = 8*sh + 2*j + rr
            y_sb = yp.tile([128, 4, 2, 4, 2, HD], BF16, tag="ysb", name=f"y_sb{b}")
            st[b]["y"] = y_sb
            for hp in range(4):
                for sh in range(2):
                    if skip_last and hp == 3 and sh == 1:
                        continue
                    s_attn_v_group(b, hp, sh)

        def s_final(b, shs=(0, 1)):
            y_sb = st[b]["y"]
            for sh in shs:
                og = ogp.tile([128, 4, 512], BF16, tag="og", name=f"og{b}_{sh}")
                for oc in range(4):
                    pf = ps_big.tile([128, C], F32, tag="big", name=f"pf{b}_{oc}_{sh}")
                    for cp in range(4):
                        nc.tensor.matmul(
                            pf[:, :],
                            w_sb["wo"][:, cp, oc * 128 : (oc + 1) * 128],
                            y_sb[:, cp, sh, :, :, :],
                            start=(cp == 0), stop=(cp == 3),
                        )
                    nc.vector.tensor_copy(og[:, oc, 0:256], pf[:, 0:256])
                    nc.scalar.copy(og[:, oc, 256:512], pf[:, 256:512])
                nc.sync.dma_start(out_d[b, :, sh, :, :], og[:, :, :])

        # ---- schedule ----
        # descriptors from all engines share the 16 DMA queues in trigger
        # order, so the early window must carry ONLY wq/wk/ident/x0; the
        # wv/wo/x1 loads are deferred by 1-element gpsimd token copies that
        # depend on qt chunks (the DMA dst overlaps the token write, so the
        # trigger inherits the dependency).
        ident = wp.tile([128, HD], BF16, tag="ident", name="ident_sb")
        _load_w("wq", wq_d, nc.scalar)
        s_load(0, nc.sync, nc.sync)
        _load_w("wk", wk_d, nc.scalar)
        nc.sync.dma_start(ident[:, :], id_d)

        wv_t = wp.tile([128, 4, C], BF16, tag="wv", name="w_wv")
        wo_t = wp.tile([128, 4, C], BF16, tag="wo", name="w_wo")
        w_sb["wv"], w_sb["wo"] = wv_t, wo_t
        x1_sb = xp.tile([128, 4, N], BF16, tag="xsb", name="x_sb1")
        st[1]["x"] = x1_sb

        def qk0_hook(ncn, qt_sb):
            if ncn == 0:
                nc.gpsimd.tensor_copy(wv_t[0:1, 0, 0:1], qt_sb[0:1, 0, 0:1])
                nc.gpsimd.dma_start(wv_t[:, :, :], wv_d[:, :, :])
            elif ncn == 2:
                nc.gpsimd.tensor_copy(wo_t[0:1, 0, 0:1], qt_sb[0:1, 2, 0:1])
                nc.gpsimd.dma_start(wo_t[:, :, :], wo_d[:, :, :])
            elif ncn == 4:
                nc.gpsimd.tensor_copy(x1_sb[0:1, 0, 0:1], qt_sb[0:1, 4, 0:1])
                nc.gpsimd.dma_start(x1_sb[:, :, :], x_d[1, :, :, :])

        st[0]["qk_hook"] = qk0_hook
        s_proj_qk(0)
        s_scores(0)
        s_proj_v(0)          # T(0) interleaved at hp==1
        s_attn_v(0)
        s_proj_qk(1)
        s_scores(1)
        s_final(0)
        s_proj_v(1)          # T(1) interleaved at hp==1
        s_attn_v(1, skip_last=True)
        s_final(1, shs=(0,))
        s_attn_v_group(1, 3, 1)
        s_final(1, shs=(1,))

    nc.compile()
    _split_excess_dma_waits(nc)
    return nc


_PROGRAM = None


def _get_program():
    global _PROGRAM
    if _PROGRAM is None:
        _PROGRAM = build_program()
    return _PROGRAM


def make_in_maps(x, Wq, Wk, Wv, Wo):
    import ml_dtypes

    bf = ml_dtypes.bfloat16
    # permute spatial axis n = 16a + r -> m = 64r + a, then lay out as
    # [128 partition, 4 cc, N] so the device DMA is a straight copy
    xm = (
        x.reshape(16, C, 64, 16)
        .transpose(0, 1, 3, 2)
        .reshape(16, 4, 128, N)
        .transpose(0, 2, 1, 3)
        .astype(bf)
    )

    def _w(W):
        return np.ascontiguousarray(
            W.T.reshape(4, 128, C).transpose(1, 0, 2).astype(bf)
        )

    wqt, wkt, wvt, wot = _w(Wq), _w(Wk), _w(Wv), _w(Wo)
    ident = np.vstack([np.eye(HD), np.eye(HD)]).astype(bf)
    in_maps = []
    for c in range(N_CORES):
        in_maps.append(
            {
                "x": np.ascontiguousarray(xm[c * B_PER_CORE : (c + 1) * B_PER_CORE]),
                "wqt": wqt,
                "wkt": wkt,
                "wvt": wvt,
                "wot": wot,
                "ident": ident,
            }
        )
    return in_maps


def kernel(x, Wq, bq, Wk, bk, Wv, bv, Wo, bo, _trace=False):
    # biases are zeros by construction in this problem (spec fill="zeros");
    # they are not applied on-device.
    nc = _get_program()
    in_maps = make_in_maps(x, Wq, Wk, Wv, Wo)
    res = run_bass_kernel_spmd(nc, in_maps, list(range(N_CORES)), trace=_trace)
    outs = [
        np.asarray(res.results[c]["out"]).astype(np.float32) for c in range(N_CORES)
    ]
    # out_r [b, p, sh, oc, s'] -> F[b, oc*128+p, sh*512+s']
    full = (
        np.concatenate(outs, axis=0)
        .transpose(0, 3, 1, 2, 4)
        .reshape(16, C, N)
        .reshape(16, C, 32, 32)
    )
    if _trace:
        return full, res
    return full
